# revision 14
# baseline (speedup 1.0000x reference)
"""Trainium2 Bass kernel for nn_Conv2d_72430328481302.

Conv2d: input (16,128,56,56) f32, weight (128,128,3,3), bias (128),
stride 1, pad 1, dilation 1 -> output (16,128,56,56).

Strategy:
  - Data-parallel over batch: 2 images per core across 8 cores, weight
    replicated.
  - The host pre-pads each image to a [Cin=128, 58*58] zero-framed
    plane so the per-image input DMA is one fully contiguous
    13.5KB-per-partition transfer (line-rate, no tiny descriptors).
  - A 3x3 stride-1 conv is then 9 accumulating float32r matmuls per
    8-output-row slab: for tap (kh,kw) the moving operand is the
    padded plane at offset (s*8+kh)*58+kw viewed as [8 rows x 56] (row
    stride 58), so the psum output is the dense 448-column slab.
    float32r runs the PE at full rate for N>=256 with ~1e-4 rel err.
  - PSUM is evacuated by the scalar engine with the bias add fused
    (Identity activation with a per-partition bias AP); output plane
    is dense [Cout, 56*56] and leaves in one contiguous DMA on the
    scalar (ACT) HWDGE ring, overlapping the input ring.
"""

import os
import sys

for _p in ("/opt/trn_rl_repo",):
    if os.path.isdir(_p) and _p not in sys.path:
        sys.path.insert(0, _p)

import numpy as np

import concourse.bass as bass
import concourse.tile as tile
from concourse import bacc, mybir
from concourse.bass_utils import run_bass_kernel_spmd

N_CORES = 8
N_IMGS = 16
IPC = N_IMGS // N_CORES  # images per core
CIN = 128
COUT = 128
H = W = 56
WP = 58  # padded width (1 col each side)
HP = 58  # padded height (1 row each side)
FLATP = HP * WP  # 3364
PAD_ALLOC = FLATP + 4  # pad to a 32B multiple
RS = 8  # output rows per slab
NSLAB = H // RS  # 7
SLAB_N = RS * W  # 448 psum columns per slab (dense)
F32 = mybir.dt.float32
F32R = mybir.dt.float32r

_CACHE = {}


def _patch_ldw_opt():
    """Enable walrus's redundant-LDWEIGHTS elimination: consecutive
    matmuls in a tap group share the stationary operand, and the f32r
    weight reload (~200ns) otherwise paces the PE stream."""
    import concourse.bass_utils as _bu

    if getattr(_bu, "_ldw_opt_patched", False):
        return
    _orig = _bu.run_command

    def _run_command(argv, **kwargs):
        argv = [
            "--enable-ldw-opt=true" if a == "--enable-ldw-opt=false" else a
            for a in argv
        ]
        return _orig(argv, **kwargs)

    _bu.run_command = _run_command
    _bu._ldw_opt_patched = True


if os.environ.get("KERNEL_LDW_OPT", "0") == "1":
    _patch_ldw_opt()


def _build_nc():
    nc = bacc.Bacc(
        "TRN2",
        target_bir_lowering=False,
        debug=False,
        num_devices=N_CORES,
    )
    x = nc.dram_tensor("x", [IPC, CIN, PAD_ALLOC], F32R, kind="ExternalInput")
    wt = nc.dram_tensor("wt", [CIN, 9, COUT], F32R, kind="ExternalInput")
    bvec = nc.dram_tensor("bvec", [COUT, 1], F32, kind="ExternalInput")
    y = nc.dram_tensor("y", [IPC, COUT, H * W], F32, kind="ExternalOutput")

    with tile.TileContext(nc) as tc:
        with (
            tc.tile_pool(name="const", bufs=1) as cpool,
            tc.tile_pool(name="xin", bufs=1) as xpool,
            tc.tile_pool(name="yout", bufs=1) as ypool,
            tc.tile_pool(name="psum", bufs=8, space="PSUM") as pspool,
        ):
            # PE warmup: a tiny scratch matmul burst during the DMA
            # lead-in trips the HAM clock gate early, so the real matmul
            # stream starts at 2.4GHz instead of paying ~9 cold matmuls
            wscr = cpool.tile([CIN, 192], F32R, name="wscr", tag="wscr")
            nc.scalar.dma_start(wscr[:], x.ap()[0, :, 0:192])

            wt_sb = cpool.tile([CIN, 9, COUT], F32R, name="wt_sb", tag="wt_sb")
            nc.sync.dma_start(wt_sb[:], wt.ap()[:])
            bias_sb = cpool.tile([COUT, 1], F32, name="bias_sb", tag="bias_sb")
            nc.scalar.dma_start(bias_sb[:], bvec.ap()[:])

            if os.environ.get("KERNEL_WARMUP", "1") == "1":
                wps = pspool.tile([COUT, 64], F32, name="ps_warm", tag="ps")
                for _ in range(14):
                    nc.tensor.matmul(
                        wps[:],
                        wscr[:, 0:128],
                        wscr[:, 128:192],
                        start=True,
                        stop=True,
                    )

            # input chunk boundaries (padded-row units); early slabs start
            # as soon as their rows (and their taps' weights) have landed
            CHUNKS = [(0, 18), (18, 42), (42, HP)]
            GROUPS = [(0, 1), (2, 3, 4), (5, 6)]

            for i in range(IPC):
                # host-padded input plane for image i
                P = xpool.tile([CIN, PAD_ALLOC], F32R, name=f"P{i}", tag=f"P{i}")
                for ci, (r0, r1) in enumerate(CHUNKS):
                    e0, e1 = r0 * WP, (r1 * WP if r1 < HP else PAD_ALLOC)
                    nc.sync.dma_start(P[:, e0:e1], x.ap()[i, :, e0:e1])

                out_sb = ypool.tile(
                    [COUT, H * W], F32, name=f"out{i}", tag=f"out{i}"
                )
                for grp in GROUPS:
                    pss = {
                        s: pspool.tile(
                            [COUT, SLAB_N], F32, name=f"ps_{i}_{s}", tag="ps"
                        )
                        for s in grp
                    }
                    # tap-outer within the group: consecutive matmuls share
                    # the stationary weights (one LDWEIGHTS per tap w/
                    # ldw-opt)
                    for t, (kh, kw) in enumerate(
                        (a, b) for a in range(3) for b in range(3)
                    ):
                        for s in grp:
                            start = (s * RS + kh) * WP + kw
                            rhs = P[:, start : start + RS * WP].rearrange(
                                "c (r k) -> c r k", k=WP
                            )[:, :, 0:W]
                            nc.tensor.matmul(
                                pss[s][:],
                                wt_sb[:, kh * 3 + kw, :],
                                rhs,
                                start=(t == 0),
                                stop=(t == 8),
                            )
                    # evacuate + fused bias add, then store the group.
                    # The final group of the final image splits its two
                    # evacs across the scalar and vector engines and
                    # stores per slab, shortening the kernel tail.
                    last_grp = i == IPC - 1 and grp == GROUPS[-1]
                    for k, s in enumerate(grp):
                        dst = out_sb[:, s * SLAB_N : (s + 1) * SLAB_N]
                        if last_grp and k % 2 == 1:
                            nc.vector.tensor_scalar_add(
                                dst, pss[s][:], bias_sb[:, :]
                            )
                        else:
                            nc.scalar.activation(
                                dst,
                                pss[s][:],
                                mybir.ActivationFunctionType.Identity,
                                bias=bias_sb[:, :],
                            )
                        if last_grp:
                            nc.sync.dma_start(
                                y.ap()[i, :, s * SLAB_N : (s + 1) * SLAB_N],
                                dst,
                            )
                    if not last_grp:
                        c0, c1 = grp[0] * SLAB_N, (grp[-1] + 1) * SLAB_N
                        nc.sync.dma_start(
                            y.ap()[i, :, c0:c1], out_sb[:, c0:c1]
                        )

    nc.compile()
    return nc


def _get_nc():
    if "nc" not in _CACHE:
        _CACHE["nc"] = _build_nc()
    return _CACHE["nc"]


def _make_in_maps(input, weight, bias):
    input = np.asarray(input)
    weight = np.asarray(weight)
    bias = np.asarray(bias)
    # pad every image into the [IPC, CIN, 58*58 (+4)] zero-framed plane
    padded = np.zeros((N_IMGS, CIN, PAD_ALLOC), dtype=np.float32)
    pv = padded[:, :, :FLATP].reshape(N_IMGS, CIN, HP, WP)
    pv[:, :, 1 : H + 1, 1 : W + 1] = input
    # weight (Cout,Cin,3,3) -> lhsT layout (Cin, kh*3+kw, Cout)
    wt_host = np.ascontiguousarray(
        np.transpose(weight, (1, 2, 3, 0)).reshape(CIN, 9, COUT), dtype=np.float32
    )
    b_host = np.ascontiguousarray(bias.reshape(COUT, 1), dtype=np.float32)
    return [
        {
            "x": padded[c * IPC : (c + 1) * IPC],
            "wt": wt_host,
            "bvec": b_host,
        }
        for c in range(N_CORES)
    ]


def run(input, weight, bias, trace=False, tmpdir=None):
    """Run the SPMD kernel; returns (output, BassKernelResults)."""
    nc = _get_nc()
    in_maps = _make_in_maps(input, weight, bias)
    res = run_bass_kernel_spmd(
        nc, in_maps, list(range(N_CORES)), trace=trace, tmpdir=tmpdir
    )
    out = np.concatenate([res.results[c]["y"] for c in range(N_CORES)], axis=0)
    return out.reshape(N_IMGS, COUT, H, W).astype(np.float32), res


def kernel(input, weight, bias):
    out, _ = run(input, weight, bias, trace=False)
    return out


# revision 15
# speedup vs baseline: 1.0132x; 1.0132x over previous
"""Trainium2 Bass kernel for nn_Conv2d_72430328481302.

Conv2d: input (16,128,56,56) f32, weight (128,128,3,3), bias (128),
stride 1, pad 1, dilation 1 -> output (16,128,56,56).

Strategy:
  - Data-parallel over batch: 2 images per core across 8 cores, weight
    replicated.
  - The host pre-pads each image to a [Cin=128, 58*58] zero-framed
    plane so the per-image input DMA is one fully contiguous
    13.5KB-per-partition transfer (line-rate, no tiny descriptors).
  - A 3x3 stride-1 conv is then 9 accumulating float32r matmuls per
    8-output-row slab: for tap (kh,kw) the moving operand is the
    padded plane at offset (s*8+kh)*58+kw viewed as [8 rows x 56] (row
    stride 58), so the psum output is the dense 448-column slab.
    float32r runs the PE at full rate for N>=256 with ~1e-4 rel err.
  - PSUM is evacuated by the scalar engine with the bias add fused
    (Identity activation with a per-partition bias AP); output plane
    is dense [Cout, 56*56] and leaves in one contiguous DMA on the
    scalar (ACT) HWDGE ring, overlapping the input ring.
"""

import os
import sys

for _p in ("/opt/trn_rl_repo",):
    if os.path.isdir(_p) and _p not in sys.path:
        sys.path.insert(0, _p)

import numpy as np

import concourse.bass as bass
import concourse.tile as tile
from concourse import bacc, mybir
from concourse.bass_utils import run_bass_kernel_spmd

N_CORES = 8
N_IMGS = 16
IPC = N_IMGS // N_CORES  # images per core
CIN = 128
COUT = 128
H = W = 56
WP = 58  # padded width (1 col each side)
HP = 58  # padded height (1 row each side)
FLATP = HP * WP  # 3364
PAD_ALLOC = FLATP + 4  # pad to a 32B multiple
RS = 8  # output rows per slab
NSLAB = H // RS  # 7
SLAB_N = RS * W  # 448 psum columns per slab (dense)
F32 = mybir.dt.float32
F32R = mybir.dt.float32r

_CACHE = {}


def _patch_ldw_opt():
    """Enable walrus's redundant-LDWEIGHTS elimination: consecutive
    matmuls in a tap group share the stationary operand, and the f32r
    weight reload (~200ns) otherwise paces the PE stream."""
    import concourse.bass_utils as _bu

    if getattr(_bu, "_ldw_opt_patched", False):
        return
    _orig = _bu.run_command

    def _run_command(argv, **kwargs):
        argv = [
            "--enable-ldw-opt=true" if a == "--enable-ldw-opt=false" else a
            for a in argv
        ]
        return _orig(argv, **kwargs)

    _bu.run_command = _run_command
    _bu._ldw_opt_patched = True


if os.environ.get("KERNEL_LDW_OPT", "0") == "1":
    _patch_ldw_opt()


def _build_nc():
    nc = bacc.Bacc(
        "TRN2",
        target_bir_lowering=False,
        debug=False,
        num_devices=N_CORES,
    )
    x = nc.dram_tensor("x", [IPC, CIN, PAD_ALLOC], F32R, kind="ExternalInput")
    wt = nc.dram_tensor("wt", [CIN, 9, COUT], F32R, kind="ExternalInput")
    bvec = nc.dram_tensor("bvec", [COUT, 1], F32, kind="ExternalInput")
    y = nc.dram_tensor("y", [IPC, COUT, H * W], F32, kind="ExternalOutput")

    with tile.TileContext(nc) as tc:
        with (
            tc.tile_pool(name="const", bufs=1) as cpool,
            tc.tile_pool(name="xin", bufs=1) as xpool,
            tc.tile_pool(name="yout", bufs=1) as ypool,
            tc.tile_pool(name="psum", bufs=8, space="PSUM") as pspool,
        ):
            wt_sb = cpool.tile([CIN, 9, COUT], F32R, name="wt_sb", tag="wt_sb")
            # first slab only needs the kh=0 taps: land them via the
            # (otherwise idle) scalar ring while the sync ring starts on
            # the remaining taps followed by the input chunks
            nc.scalar.dma_start(wt_sb[:, 0:3, :], wt.ap()[:, 0:3, :])
            nc.sync.dma_start(wt_sb[:, 3:9, :], wt.ap()[:, 3:9, :])
            bias_sb = cpool.tile([COUT, 1], F32, name="bias_sb", tag="bias_sb")
            nc.scalar.dma_start(bias_sb[:], bvec.ap()[:])

            # input chunk boundaries (padded-row units); early slabs start
            # as soon as their rows (and their taps' weights) have landed
            CHUNKS = [(0, 18), (18, 42), (42, HP)]
            GROUPS = [(0, 1), (2, 3, 4), (5, 6)]

            for i in range(IPC):
                # host-padded input plane for image i
                P = xpool.tile([CIN, PAD_ALLOC], F32R, name=f"P{i}", tag=f"P{i}")
                for ci, (r0, r1) in enumerate(CHUNKS):
                    e0, e1 = r0 * WP, (r1 * WP if r1 < HP else PAD_ALLOC)
                    nc.sync.dma_start(P[:, e0:e1], x.ap()[i, :, e0:e1])

                out_sb = ypool.tile(
                    [COUT, H * W], F32, name=f"out{i}", tag=f"out{i}"
                )
                for grp in GROUPS:
                    pss = {
                        s: pspool.tile(
                            [COUT, SLAB_N], F32, name=f"ps_{i}_{s}", tag="ps"
                        )
                        for s in grp
                    }
                    # tap-outer within the group: consecutive matmuls share
                    # the stationary weights (one LDWEIGHTS per tap w/
                    # ldw-opt)
                    for t, (kh, kw) in enumerate(
                        (a, b) for a in range(3) for b in range(3)
                    ):
                        for s in grp:
                            start = (s * RS + kh) * WP + kw
                            rhs = P[:, start : start + RS * WP].rearrange(
                                "c (r k) -> c r k", k=WP
                            )[:, :, 0:W]
                            nc.tensor.matmul(
                                pss[s][:],
                                wt_sb[:, kh * 3 + kw, :],
                                rhs,
                                start=(t == 0),
                                stop=(t == 8),
                            )
                    # evacuate + fused bias add, then store the group.
                    # The final group of the final image splits its two
                    # evacs across the scalar and vector engines and
                    # stores per slab, shortening the kernel tail.
                    last_grp = i == IPC - 1 and grp == GROUPS[-1]
                    for k, s in enumerate(grp):
                        dst = out_sb[:, s * SLAB_N : (s + 1) * SLAB_N]
                        if last_grp and k % 2 == 1:
                            nc.vector.tensor_scalar_add(
                                dst, pss[s][:], bias_sb[:, :]
                            )
                        else:
                            nc.scalar.activation(
                                dst,
                                pss[s][:],
                                mybir.ActivationFunctionType.Identity,
                                bias=bias_sb[:, :],
                            )
                        if last_grp:
                            nc.sync.dma_start(
                                y.ap()[i, :, s * SLAB_N : (s + 1) * SLAB_N],
                                dst,
                            )
                    if not last_grp:
                        c0, c1 = grp[0] * SLAB_N, (grp[-1] + 1) * SLAB_N
                        nc.sync.dma_start(
                            y.ap()[i, :, c0:c1], out_sb[:, c0:c1]
                        )

    nc.compile()
    return nc


def _get_nc():
    if "nc" not in _CACHE:
        _CACHE["nc"] = _build_nc()
    return _CACHE["nc"]


def _make_in_maps(input, weight, bias):
    input = np.asarray(input)
    weight = np.asarray(weight)
    bias = np.asarray(bias)
    # pad every image into the [IPC, CIN, 58*58 (+4)] zero-framed plane
    padded = np.zeros((N_IMGS, CIN, PAD_ALLOC), dtype=np.float32)
    pv = padded[:, :, :FLATP].reshape(N_IMGS, CIN, HP, WP)
    pv[:, :, 1 : H + 1, 1 : W + 1] = input
    # weight (Cout,Cin,3,3) -> lhsT layout (Cin, kh*3+kw, Cout)
    wt_host = np.ascontiguousarray(
        np.transpose(weight, (1, 2, 3, 0)).reshape(CIN, 9, COUT), dtype=np.float32
    )
    b_host = np.ascontiguousarray(bias.reshape(COUT, 1), dtype=np.float32)
    return [
        {
            "x": padded[c * IPC : (c + 1) * IPC],
            "wt": wt_host,
            "bvec": b_host,
        }
        for c in range(N_CORES)
    ]


def run(input, weight, bias, trace=False, tmpdir=None):
    """Run the SPMD kernel; returns (output, BassKernelResults)."""
    nc = _get_nc()
    in_maps = _make_in_maps(input, weight, bias)
    res = run_bass_kernel_spmd(
        nc, in_maps, list(range(N_CORES)), trace=trace, tmpdir=tmpdir
    )
    out = np.concatenate([res.results[c]["y"] for c in range(N_CORES)], axis=0)
    return out.reshape(N_IMGS, COUT, H, W).astype(np.float32), res


def kernel(input, weight, bias):
    out, _ = run(input, weight, bias, trace=False)
    return out
